# revision 8
# baseline (speedup 1.0000x reference)
"""BlurDegradation kernel for 8x TRN2 NeuronCores.

Math: t[b] successive 11x11 depthwise *circular* convolutions compose into a
single circular convolution whose spectrum is the product of the per-step
spectra. The host composes the (tiny) 20 step-kernels into 21 cumulative
spectra with numpy FFTs and selects per-sample spectrum FK[t[b]]; each device
then computes, per image,  out = Re( F* . (FK o (F x F)) . F* ) / N^2  as four
chained dense matmul stages on the PE array (plus a DVE pointwise complex
multiply). Using the previous stage's output as the *stationary* (lhsT)
operand makes each stage implicitly transpose, so no explicit transposes are
needed.

Half-spectrum: spectra of real fields are Hermitian, so only k-rows 0..256 are
needed; the Hermitian fold weights (1,2,...,2,1) and the 1/N^2 scale are
folded into FK on the host. fp32r matmuls require even N and full col-groups,
so the k-dim is padded to 384 (3 full 128-tiles); FK rows 257..383 are zero,
which also zeroes the garbage H rows during the pointwise multiply.

Sharding: pure data parallel, 8 samples per core, no cross-core comms.
"""

import numpy as np

N = 512
P = 128
T_STEPS = 20
KS = 11
KROWS = N // 2 + 1  # 257 informative k-rows
KPAD = 384          # padded k-dim: 3 full partition tiles
KT = KPAD // P      # 3
NCORES = 8
BATCH = 64
CHANNELS = 3
SPC = BATCH // NCORES  # samples per core
IMGS = SPC * CHANNELS  # images per core

# float32r: PE runs 4-byte matmuls at full rate (1 cyc/row) with ~tf32
# mantissa; float32 is exact but 4 cyc/row.
USE_F32R = True

_PROGRAM = None
TRACE = False
LAST_EXEC_NS = None
LAST_TRACE = None


def _build_program():
    import concourse.mybir as mybir
    import concourse.tile as tile
    from concourse import bacc

    f32 = mybir.dt.float32
    f32r = mybir.dt.float32r
    mmdt = f32r if USE_F32R else f32

    nc = bacc.Bacc(
        "TRN2", target_bir_lowering=False, debug=False, num_devices=NCORES
    )
    x_d = nc.dram_tensor("x", [IMGS, N, N], mmdt, kind="ExternalInput").ap()
    fkr_d = nc.dram_tensor("fkr", [SPC, KPAD, N], f32, kind="ExternalInput").ap()
    fki_d = nc.dram_tensor("fki", [SPC, KPAD, N], f32, kind="ExternalInput").ap()
    cmat_d = nc.dram_tensor("cmat", [N, N], mmdt, kind="ExternalInput").ap()
    smat_d = nc.dram_tensor("smat", [N, N], mmdt, kind="ExternalInput").ap()
    snmat_d = nc.dram_tensor("snmat", [N, N], mmdt, kind="ExternalInput").ap()
    out_d = nc.dram_tensor("out", [IMGS, N, N], f32, kind="ExternalOutput").ap()

    with tile.TileContext(nc) as tc:
        with (
            tc.tile_pool(name="mats", bufs=1) as mats,
            tc.tile_pool(name="xsp", bufs=2) as xsp,
            tc.tile_pool(name="outp", bufs=2) as outp,
            tc.tile_pool(name="fkp", bufs=2) as fkp,
            tc.tile_pool(name="ap_", bufs=2) as ap_,
            tc.tile_pool(name="zp", bufs=2) as zp,
            tc.tile_pool(name="wp", bufs=2) as wp,
            tc.tile_pool(name="pw", bufs=2) as pw,
            tc.tile_pool(name="psum", bufs=8, space="PSUM") as psum,
        ):
            # resident DFT matrices, [p, ytile, n] layout
            Cs = mats.tile([P, 4, N], mmdt)
            Ss = mats.tile([P, 4, N], mmdt)
            Sns = mats.tile([P, 4, N], mmdt)
            nc.sync.dma_start(Cs[:], cmat_d.rearrange("(i p) n -> p i n", p=P))
            nc.sync.dma_start(Ss[:], smat_d.rearrange("(i p) n -> p i n", p=P))
            nc.sync.dma_start(Sns[:], snmat_d.rearrange("(i p) n -> p i n", p=P))

            for s in range(SPC):
                # per-sample spectrum, k-rows 0..383 (257..383 are zero)
                fkr = fkp.tile([P, KT, N], f32, tag="fkr")
                fki = fkp.tile([P, KT, N], f32, tag="fki")
                nc.sync.dma_start(
                    fkr[:], fkr_d[s].rearrange("(i p) n -> p i n", p=P)
                )
                nc.sync.dma_start(
                    fki[:], fki_d[s].rearrange("(i p) n -> p i n", p=P)
                )

                for ch in range(CHANNELS):
                    img = s * CHANNELS + ch
                    xs = xsp.tile([P, 4, N], mmdt, tag="xs")
                    nc.sync.dma_start(
                        xs[:], x_d[img].rearrange("(i p) n -> p i n", p=P)
                    )

                    # ---- Stage 1: A = x^T F  (cols k=0..383)
                    # A_r = x^T C[:, :384] ; A_i = x^T (-S)[:, :384]
                    Ar = ap_.tile([P, 4, KPAD], mmdt, tag="Ar")
                    Ai = ap_.tile([P, 4, KPAD], mmdt, tag="Ai")
                    for m in range(4):
                        pa = psum.tile([P, N], f32, tag="ps", name="pa")[:, :KPAD]
                        for kk in range(4):
                            nc.tensor.matmul(
                                pa[:],
                                xs[:, kk, m * P : (m + 1) * P],
                                Cs[:, kk, 0:KPAD],
                                start=(kk == 0),
                                stop=(kk == 3),
                            )
                        nc.any.tensor_copy(out=Ar[:, m, :], in_=pa[:])
                        pb = psum.tile([P, N], f32, tag="ps", name="pb")[:, :KPAD]
                        for kk in range(4):
                            nc.tensor.matmul(
                                pb[:],
                                xs[:, kk, m * P : (m + 1) * P],
                                Sns[:, kk, 0:KPAD],
                                start=(kk == 0),
                                stop=(kk == 3),
                            )
                        nc.any.tensor_copy(out=Ai[:, m, :], in_=pb[:])

                    # ---- Stage 2: H = A^T F  (k-rows 0..383), fused pointwise
                    # H_r = A_r^T C + A_i^T S ; H_i = A_i^T C + A_r^T (-S)
                    # Z = H o FK   (FK rows 257..383 are zero)
                    Zr = zp.tile([P, KT, N], mmdt, tag="Zr")
                    Zi = zp.tile([P, KT, N], mmdt, tag="Zi")
                    for km in range(KT):
                        ksl = slice(km * P, (km + 1) * P)
                        ph_r = psum.tile([P, N], f32, tag="ps", name="ph_r")
                        for kk in range(4):
                            nc.tensor.matmul(
                                ph_r[:],
                                Ar[:, kk, ksl],
                                Cs[:, kk, :],
                                start=(kk == 0),
                                stop=False,
                            )
                        for kk in range(4):
                            nc.tensor.matmul(
                                ph_r[:],
                                Ai[:, kk, ksl],
                                Ss[:, kk, :],
                                start=False,
                                stop=(kk == 3),
                            )
                        ph_i = psum.tile([P, N], f32, tag="ps", name="ph_i")
                        for kk in range(4):
                            nc.tensor.matmul(
                                ph_i[:],
                                Ai[:, kk, ksl],
                                Cs[:, kk, :],
                                start=(kk == 0),
                                stop=False,
                            )
                        for kk in range(4):
                            nc.tensor.matmul(
                                ph_i[:],
                                Ar[:, kk, ksl],
                                Sns[:, kk, :],
                                start=False,
                                stop=(kk == 3),
                            )
                        fr = fkr[:, km, :]
                        fi = fki[:, km, :]
                        zr = Zr[:, km, :]
                        zi = Zi[:, km, :]
                        t1 = pw.tile([P, N], f32, tag="t1")
                        t2 = pw.tile([P, N], f32, tag="t2")
                        nc.vector.tensor_mul(out=zr, in0=ph_r[:], in1=fr)
                        nc.vector.tensor_mul(out=t1[:], in0=ph_i[:], in1=fi)
                        nc.vector.tensor_sub(out=zr, in0=zr, in1=t1[:])
                        nc.vector.tensor_mul(out=zi, in0=ph_r[:], in1=fi)
                        nc.vector.tensor_mul(out=t2[:], in0=ph_i[:], in1=fr)
                        nc.vector.tensor_add(out=zi, in0=zi, in1=t2[:])

                    # ---- Stage 3: W[l,y] = sum_k Z[k,l] F*[k,y]
                    # W_r = Z_r^T C + Z_i^T (-S) ; W_i = Z_r^T S + Z_i^T C
                    Wr = wp.tile([P, 4, N], mmdt, tag="Wr")
                    Wi = wp.tile([P, 4, N], mmdt, tag="Wi")
                    for lm in range(4):
                        lsl = slice(lm * P, (lm + 1) * P)
                        pw_r = psum.tile([P, N], f32, tag="ps", name="pw_r")
                        for kt in range(KT):
                            nc.tensor.matmul(
                                pw_r[:],
                                Zr[:, kt, lsl],
                                Cs[:, kt, :],
                                start=(kt == 0),
                                stop=False,
                            )
                        for kt in range(KT):
                            nc.tensor.matmul(
                                pw_r[:],
                                Zi[:, kt, lsl],
                                Sns[:, kt, :],
                                start=False,
                                stop=(kt == KT - 1),
                            )
                        nc.any.tensor_copy(out=Wr[:, lm, :], in_=pw_r[:])
                        pw_i = psum.tile([P, N], f32, tag="ps", name="pw_i")
                        for kt in range(KT):
                            nc.tensor.matmul(
                                pw_i[:],
                                Zr[:, kt, lsl],
                                Ss[:, kt, :],
                                start=(kt == 0),
                                stop=False,
                            )
                        for kt in range(KT):
                            nc.tensor.matmul(
                                pw_i[:],
                                Zi[:, kt, lsl],
                                Cs[:, kt, :],
                                start=False,
                                stop=(kt == KT - 1),
                            )
                        nc.any.tensor_copy(out=Wi[:, lm, :], in_=pw_i[:])

                    # ---- Stage 4: out[y,x] = Re(sum_l W[l,y] F*[l,x])
                    # out = W_r^T C + W_i^T (-S)
                    outs = outp.tile([P, 4, N], f32, tag="outs")
                    for ym in range(4):
                        ysl = slice(ym * P, (ym + 1) * P)
                        po = psum.tile([P, N], f32, tag="ps", name="po")
                        for lt in range(4):
                            nc.tensor.matmul(
                                po[:],
                                Wr[:, lt, ysl],
                                Cs[:, lt, :],
                                start=(lt == 0),
                                stop=False,
                            )
                        for lt in range(4):
                            nc.tensor.matmul(
                                po[:],
                                Wi[:, lt, ysl],
                                Sns[:, lt, :],
                                start=False,
                                stop=(lt == 3),
                            )
                        nc.any.tensor_copy(out=outs[:, ym, :], in_=po[:])
                    nc.sync.dma_start(
                        out_d[img].rearrange("(i p) n -> p i n", p=P), outs[:]
                    )

    nc.compile()
    return nc


def _host_spectra(kernels):
    """Compose step kernels into 21 cumulative half-spectra with Hermitian
    weights and 1/N^2 folded in, zero-padded to KPAD k-rows.
    Returns (FKr, FKi) f32 [21, KPAD, 512]."""
    kernels = np.asarray(kernels, dtype=np.float64)
    h = np.zeros((T_STEPS, N, N), np.float64)
    idx = (KS // 2 - np.arange(KS)) % N
    h[:, idx[:, None], idx[None, :]] = kernels
    s_step = np.fft.fft2(h)
    cum = np.ones((T_STEPS + 1, N, N), np.complex128)
    for i in range(1, T_STEPS + 1):
        cum[i] = cum[i - 1] * s_step[i - 1]
    w = np.zeros(KPAD)
    w[:KROWS] = 2.0
    w[0] = w[KROWS - 1] = 1.0
    fk = cum[:, :KPAD, :] * w[None, :, None] / float(N * N)
    return fk.real.astype(np.float32), fk.imag.astype(np.float32)


def _dft_mats():
    j = np.arange(N)
    ang = 2.0 * np.pi * (np.outer(j, j) % N) / N
    cm = np.cos(ang).astype(np.float32)
    sm = np.sin(ang).astype(np.float32)
    return cm, sm, np.ascontiguousarray(-sm)


def kernel(x0, t, kernels):
    global _PROGRAM
    from concourse import bass_utils

    x0 = np.ascontiguousarray(np.asarray(x0), dtype=np.float32)
    tt = np.asarray(t).astype(np.int64)
    fkr_all, fki_all = _host_spectra(kernels)
    cm, sm, snm = _dft_mats()

    if _PROGRAM is None:
        _PROGRAM = _build_program()
    nc = _PROGRAM

    in_maps = []
    for c in range(NCORES):
        sl = slice(c * SPC, (c + 1) * SPC)
        ts = tt[sl]
        in_maps.append(
            {
                "x": np.ascontiguousarray(x0[sl].reshape(IMGS, N, N)),
                "fkr": np.ascontiguousarray(fkr_all[ts]),
                "fki": np.ascontiguousarray(fki_all[ts]),
                "cmat": cm,
                "smat": sm,
                "snmat": snm,
            }
        )

    global LAST_EXEC_NS, LAST_TRACE
    res = bass_utils.run_bass_kernel_spmd(
        nc, in_maps, core_ids=list(range(NCORES)), trace=TRACE
    )
    LAST_EXEC_NS = res.exec_time_ns
    if res.instructions_and_trace is not None:
        LAST_TRACE = res.instructions_and_trace[1]
    out = np.empty((BATCH, CHANNELS, N, N), np.float32)
    for c in range(NCORES):
        out[c * SPC : (c + 1) * SPC] = res.results[c]["out"].reshape(
            SPC, CHANNELS, N, N
        )
    return out


# revision 11
# speedup vs baseline: 1.0548x; 1.0548x over previous
"""BlurDegradation kernel for 8x TRN2 NeuronCores.

Math: t[b] successive 11x11 depthwise *circular* convolutions compose into a
single circular convolution whose spectrum is the product of the per-step
spectra. The host composes the (tiny) 20 step-kernels into 21 cumulative
spectra with numpy FFTs and selects per-sample spectrum FK[t[b]]; each device
then computes, per image,  out = Re( F* . (FK o (F x F)) . F* ) / N^2  as four
chained dense matmul stages on the PE array (plus DVE pointwise work). Using
the previous stage's output as one matmul operand with the contraction on the
partition dim makes each stage implicitly transpose, so no PE/DMA transposes
are needed. The device emits the final image transposed ([x,y]); the host
swaps the last two axes after the gather.

Row-count optimizations vs the naive 4-stage dense chain:
 - Hermitian half-spectrum: y-frequencies k=0..256 only (pad to 258 for the
   fp32r even-moving-dim rule); the fold weights (1,2,..,2,1,0) and 1/N^2 are
   folded into FK on the host.
 - Gauss 3-mult complex multiplies in stages 2 and 3 (3 matmul chains instead
   of 4); the +/- recombines ride the existing PSUM-evacuation DVE ops.
 - Stage 2 is matrix-stationary so the k-half axis is the cheap *moving* dim
   (N=258); stages 3/4 keep data-stationary form, with the k=256..257 rows
   handled by an overlapped third M-tile (130:258) and a K=2 tail matmul.

Sharding: pure data parallel, 8 samples per core, no cross-core comms.
"""

import numpy as np

N = 512
P = 128
T_STEPS = 20
KS = 11
KP = 258            # padded half-spectrum k-dim (even for fp32r)
NCORES = 8
BATCH = 64
CHANNELS = 3
SPC = BATCH // NCORES  # samples per core
IMGS = SPC * CHANNELS  # images per core

# float32r: PE runs 4-byte matmuls at full rate (1 cyc/row) with ~tf32
# mantissa; float32 is exact but 4 cyc/row.
USE_F32R = True

_PROGRAM = None
TRACE = False
LAST_EXEC_NS = None
LAST_TRACE = None


def _build_program():
    import concourse.mybir as mybir
    import concourse.tile as tile
    from concourse import bacc

    f32 = mybir.dt.float32
    f32r = mybir.dt.float32r
    mmdt = f32r if USE_F32R else f32

    nc = bacc.Bacc(
        "TRN2", target_bir_lowering=False, debug=False, num_devices=NCORES
    )
    x_d = nc.dram_tensor("x", [IMGS, N, N], mmdt, kind="ExternalInput").ap()
    fkr_d = nc.dram_tensor("fkr", [SPC, N, KP], f32, kind="ExternalInput").ap()
    fki_d = nc.dram_tensor("fki", [SPC, N, KP], f32, kind="ExternalInput").ap()
    # DFT matrix combos (all symmetric 512x512): C, S, -S, -(S+C), C-S, C+S
    mat_names = ["cmat", "smat", "snmat", "nscmat", "cmsmat", "cpsmat"]
    mat_d = {
        nm: nc.dram_tensor(nm, [N, N], mmdt, kind="ExternalInput").ap()
        for nm in mat_names
    }
    out_d = nc.dram_tensor("out", [IMGS, N, N], f32, kind="ExternalOutput").ap()

    with tile.TileContext(nc) as tc:
        with (
            tc.tile_pool(name="mats", bufs=1) as mats,
            tc.tile_pool(name="xsp", bufs=2) as xsp,
            tc.tile_pool(name="outp", bufs=2) as outp,
            tc.tile_pool(name="fkp", bufs=2) as fkp,
            tc.tile_pool(name="apool", bufs=2) as apool,
            tc.tile_pool(name="zpool", bufs=2) as zpool,
            tc.tile_pool(name="vpool", bufs=2) as vpool,
            tc.tile_pool(name="pw", bufs=2) as pw,
            tc.tile_pool(name="psum", bufs=8, space="PSUM") as psum,
        ):
            # resident DFT matrices, [p, tile, n] layout
            M = {}
            for nm in mat_names:
                mt = mats.tile([P, 4, N], mmdt, name=nm + "_s")
                nc.sync.dma_start(mt[:], mat_d[nm].rearrange("(i p) n -> p i n", p=P))
                M[nm] = mt
            Cs, Ss, Sns = M["cmat"], M["smat"], M["snmat"]
            nSCs, CmSs, CpSs = M["nscmat"], M["cmsmat"], M["cpsmat"]
            # C/-S rows 130..257, partition-aligned (for the stage-4 k tail)
            Ck2s = mats.tile([P, N], mmdt, name="ck2_s")
            Snk2s = mats.tile([P, N], mmdt, name="snk2_s")
            nc.sync.dma_start(Ck2s[:], mat_d["cmat"][130:258, :])
            nc.sync.dma_start(Snk2s[:], mat_d["snmat"][130:258, :])

            for s in range(SPC):
                # per-sample spectrum, transposed [l, k] layout, k cols 0..257
                fktr = fkp.tile([P, 4, KP], f32, tag="fktr")
                fkti = fkp.tile([P, 4, KP], f32, tag="fkti")
                nc.sync.dma_start(
                    fktr[:], fkr_d[s].rearrange("(i p) n -> p i n", p=P)
                )
                nc.sync.dma_start(
                    fkti[:], fki_d[s].rearrange("(i p) n -> p i n", p=P)
                )

                for ch in range(CHANNELS):
                    img = s * CHANNELS + ch
                    xs = xsp.tile([P, 4, N], mmdt, tag="xs")
                    nc.sync.dma_start(
                        xs[:], x_d[img].rearrange("(i p) n -> p i n", p=P)
                    )

                    # ---- Stage 1: A = x^T F  (k cols 0..257)
                    # A_r = x^T C[:, :258] ; A_i = x^T (-S)[:, :258] ; Apb = A_r + A_i
                    Ar = apool.tile([P, 4, KP], mmdt, tag="Ar")
                    Ai = apool.tile([P, 4, KP], mmdt, tag="Ai")
                    Apb = apool.tile([P, 4, KP], mmdt, tag="Apb")
                    for m in range(4):
                        pa = psum.tile([P, N], f32, tag="ps", name="pa")[:, :KP]
                        for kk in range(4):
                            nc.tensor.matmul(
                                pa[:],
                                xs[:, kk, m * P : (m + 1) * P],
                                Cs[:, kk, 0:KP],
                                start=(kk == 0),
                                stop=(kk == 3),
                            )
                        pb = psum.tile([P, N], f32, tag="ps", name="pb")[:, :KP]
                        for kk in range(4):
                            nc.tensor.matmul(
                                pb[:],
                                xs[:, kk, m * P : (m + 1) * P],
                                Sns[:, kk, 0:KP],
                                start=(kk == 0),
                                stop=(kk == 3),
                            )
                        nc.any.tensor_copy(out=Ar[:, m, :], in_=pa[:])
                        nc.any.tensor_copy(out=Ai[:, m, :], in_=pb[:])
                        nc.vector.tensor_add(
                            out=Apb[:, m, :], in0=Ar[:, m, :], in1=pb[:]
                        )

                    # ---- Stage 2 (matrix-stationary): Ht[l,k] = (F A)[l,k]
                    # Gauss: m1 = C.Apb ; m2 = (-S-C).Ar ; m3 = (C-S).Ai
                    # Htr = m1 - m3 ; Hti = m1 + m2
                    # pointwise: Zt = Ht o FKt ; store Ztr, Zq=Zti-Ztr, Zp=Ztr+Zti
                    Ztr = zpool.tile([P, 4, KP], mmdt, tag="Ztr")
                    Zq = zpool.tile([P, 4, KP], mmdt, tag="Zq")
                    Zp = zpool.tile([P, 4, KP], mmdt, tag="Zp")
                    for lm in range(4):
                        lsl = slice(lm * P, (lm + 1) * P)
                        m1 = psum.tile([P, N], f32, tag="ps", name="m1")[:, :KP]
                        m2 = psum.tile([P, N], f32, tag="ps", name="m2")[:, :KP]
                        m3 = psum.tile([P, N], f32, tag="ps", name="m3")[:, :KP]
                        for kk in range(4):
                            nc.tensor.matmul(
                                m1[:], Cs[:, kk, lsl], Apb[:, kk, :],
                                start=(kk == 0), stop=(kk == 3),
                            )
                        for kk in range(4):
                            nc.tensor.matmul(
                                m2[:], nSCs[:, kk, lsl], Ar[:, kk, :],
                                start=(kk == 0), stop=(kk == 3),
                            )
                        for kk in range(4):
                            nc.tensor.matmul(
                                m3[:], CmSs[:, kk, lsl], Ai[:, kk, :],
                                start=(kk == 0), stop=(kk == 3),
                            )
                        fr = fktr[:, lm, :]
                        fi = fkti[:, lm, :]
                        hr = pw.tile([P, KP], f32, tag="hr")
                        hi = pw.tile([P, KP], f32, tag="hi")
                        zti = pw.tile([P, KP], f32, tag="zti")
                        tt = pw.tile([P, KP], f32, tag="tt")
                        m1s = pw.tile([P, KP], f32, tag="m1s")
                        nc.any.tensor_copy(out=m1s[:], in_=m1[:])
                        nc.vector.tensor_sub(out=hr[:], in0=m1s[:], in1=m3[:])
                        nc.vector.tensor_add(out=hi[:], in0=m1s[:], in1=m2[:])
                        ztr = Ztr[:, lm, :]
                        nc.vector.tensor_mul(out=ztr, in0=hr[:], in1=fr)
                        nc.vector.tensor_mul(out=tt[:], in0=hi[:], in1=fi)
                        nc.vector.tensor_sub(out=ztr, in0=ztr, in1=tt[:])
                        nc.vector.tensor_mul(out=zti[:], in0=hr[:], in1=fi)
                        nc.vector.tensor_mul(out=tt[:], in0=hi[:], in1=fr)
                        nc.vector.tensor_add(out=zti[:], in0=zti[:], in1=tt[:])
                        nc.vector.tensor_sub(out=Zq[:, lm, :], in0=zti[:], in1=ztr)
                        nc.vector.tensor_add(out=Zp[:, lm, :], in0=ztr, in1=zti[:])

                    # ---- Stage 3: V[k,x] = sum_l Zt[l,k] F*[l,x]
                    # Gauss: n1 = Ztr^T (C+S) ; n2 = Zq^T C ; n3 = Zp^T S
                    # V_r = n1 - n3 ; V_i = n1 + n2
                    # k M-tiles: 0:128, 128:256, 130:258 (rows 126:128 new)
                    Vr = vpool.tile([P, 3, N], mmdt, tag="Vr")
                    Vi = vpool.tile([P, 3, N], mmdt, tag="Vi")
                    for km in range(3):
                        koff = (0, 128, 130)[km]
                        ksl = slice(koff, koff + P)
                        n1 = psum.tile([P, N], f32, tag="ps", name="n1")
                        n2 = psum.tile([P, N], f32, tag="ps", name="n2")
                        n3 = psum.tile([P, N], f32, tag="ps", name="n3")
                        for lt in range(4):
                            nc.tensor.matmul(
                                n1[:], Ztr[:, lt, ksl], CpSs[:, lt, :],
                                start=(lt == 0), stop=(lt == 3),
                            )
                        for lt in range(4):
                            nc.tensor.matmul(
                                n2[:], Zq[:, lt, ksl], Cs[:, lt, :],
                                start=(lt == 0), stop=(lt == 3),
                            )
                        for lt in range(4):
                            nc.tensor.matmul(
                                n3[:], Zp[:, lt, ksl], Ss[:, lt, :],
                                start=(lt == 0), stop=(lt == 3),
                            )
                        n1s = pw.tile([P, N], f32, tag="n1s")
                        nc.any.tensor_copy(out=n1s[:], in_=n1[:])
                        nc.vector.tensor_sub(
                            out=Vr[:, km, :], in0=n1s[:], in1=n3[:]
                        )
                        nc.vector.tensor_add(
                            out=Vi[:, km, :], in0=n1s[:], in1=n2[:]
                        )

                    # ---- Stage 4: outT[x,y] = Re(sum_k V[k,x] F*[k,y])
                    # outT = V_r^T C[k-rows] + V_i^T (-S)[k-rows]; K tiles 128,128,2
                    outs = outp.tile([P, 4, N], f32, tag="outs")
                    for xm in range(4):
                        xsl = slice(xm * P, (xm + 1) * P)
                        po = psum.tile([P, N], f32, tag="ps", name="po")
                        # k-coverage: tile0 = 0..127, tile1 rows 0..1 =
                        # 128..129, tile2 = 130..257 (C/S rows must match)
                        nc.tensor.matmul(
                            po[:], Vr[:, 0, xsl], Cs[:, 0, :],
                            start=True, stop=False,
                        )
                        nc.tensor.matmul(
                            po[:], Vr[0:2, 1, xsl], Cs[0:2, 1, :],
                            start=False, stop=False,
                        )
                        nc.tensor.matmul(
                            po[:], Vr[:, 2, xsl], Ck2s[:, :],
                            start=False, stop=False,
                        )
                        nc.tensor.matmul(
                            po[:], Vi[:, 0, xsl], Sns[:, 0, :],
                            start=False, stop=False,
                        )
                        nc.tensor.matmul(
                            po[:], Vi[0:2, 1, xsl], Sns[0:2, 1, :],
                            start=False, stop=False,
                        )
                        nc.tensor.matmul(
                            po[:], Vi[:, 2, xsl], Snk2s[:, :],
                            start=False, stop=True,
                        )
                        nc.any.tensor_copy(out=outs[:, xm, :], in_=po[:])
                    nc.sync.dma_start(
                        out_d[img].rearrange("(i p) n -> p i n", p=P), outs[:]
                    )

    nc.compile()
    return nc


def _host_spectra(kernels):
    """Compose step kernels into 21 cumulative half-spectra, transposed to
    [l, k] layout with Hermitian weights and 1/N^2 folded in.
    Returns (FKtr, FKti) f32 [21, 512, KP]."""
    kernels = np.asarray(kernels, dtype=np.float64)
    h = np.zeros((T_STEPS, N, N), np.float64)
    idx = (KS // 2 - np.arange(KS)) % N
    h[:, idx[:, None], idx[None, :]] = kernels
    s_step = np.fft.fft2(h)
    cum = np.ones((T_STEPS + 1, N, N), np.complex128)
    for i in range(1, T_STEPS + 1):
        cum[i] = cum[i - 1] * s_step[i - 1]
    w = np.zeros(KP)
    w[: N // 2 + 1] = 2.0
    w[0] = w[N // 2] = 1.0
    fkt = (cum[:, :KP, :] * w[None, :, None] / float(N * N)).transpose(0, 2, 1)
    return (
        np.ascontiguousarray(fkt.real.astype(np.float32)),
        np.ascontiguousarray(fkt.imag.astype(np.float32)),
    )


def _dft_mats():
    j = np.arange(N)
    ang = 2.0 * np.pi * (np.outer(j, j) % N) / N
    cm = np.cos(ang).astype(np.float32)
    sm = np.sin(ang).astype(np.float32)
    return {
        "cmat": cm,
        "smat": sm,
        "snmat": np.ascontiguousarray(-sm),
        "nscmat": np.ascontiguousarray(-sm - cm),
        "cmsmat": np.ascontiguousarray(cm - sm),
        "cpsmat": np.ascontiguousarray(cm + sm),
    }


def kernel(x0, t, kernels):
    global _PROGRAM, LAST_EXEC_NS, LAST_TRACE
    from concourse import bass_utils

    x0 = np.ascontiguousarray(np.asarray(x0), dtype=np.float32)
    tt = np.asarray(t).astype(np.int64)
    fktr_all, fkti_all = _host_spectra(kernels)
    mats = _dft_mats()

    if _PROGRAM is None:
        _PROGRAM = _build_program()
    nc = _PROGRAM

    in_maps = []
    for c in range(NCORES):
        sl = slice(c * SPC, (c + 1) * SPC)
        ts = tt[sl]
        im = {
            "x": np.ascontiguousarray(x0[sl].reshape(IMGS, N, N)),
            "fkr": np.ascontiguousarray(fktr_all[ts]),
            "fki": np.ascontiguousarray(fkti_all[ts]),
        }
        im.update(mats)
        in_maps.append(im)

    res = bass_utils.run_bass_kernel_spmd(
        nc, in_maps, core_ids=list(range(NCORES)), trace=TRACE
    )
    LAST_EXEC_NS = res.exec_time_ns
    if res.instructions_and_trace is not None:
        LAST_TRACE = res.instructions_and_trace[1]
    out = np.empty((BATCH, CHANNELS, N, N), np.float32)
    for c in range(NCORES):
        # device emits [x, y]; swap back to [y, x]
        out[c * SPC : (c + 1) * SPC] = (
            res.results[c]["out"]
            .reshape(SPC, CHANNELS, N, N)
            .transpose(0, 1, 3, 2)
        )
    return out
